# revision 10
# baseline (speedup 1.0000x reference)
"""RBF kernel regression (Gauss transform) on 8 Trainium2 NeuronCores.

Computes out = K @ alpha where K[b, n] = exp(-||z_b - x_n||^2 / 2),
z: [2048, 64], dataset: [100000, 64], alpha: [100000, 16].

Strategy (sharding_hint): shard dataset/alpha row-wise (N) across 8 cores.
Each core computes partial[f, b] = sum_n alpha[n, f] * G[n, b] with
G = exp(z.x_n - 0.5*||x_n||^2), and the host applies the remaining
exp(-0.5*||z_b||^2) factor, sums partials over cores, and transposes.

Per-core device pipeline (all operands pre-packed/transposed on host):
  for each n-tile (128 rows) and b-half (1024 cols):
    cross = dsT_tile^T @ zT          (TensorE, float32r fast mode)
    G     = exp(cross + bias[n])     (ScalarE, per-partition bias = -0.5*x^2)
    acc  += alpha_tile^T @ G         (TensorE, accumulating PSUM group)
"""

import sys

if "/opt/trn_rl_repo" not in sys.path:
    sys.path.insert(0, "/opt/trn_rl_repo")

import numpy as np

B = 2048  # batch (queries)
D = 64  # feature dim
F = 16  # output dim
NCORES = 8
N_FULL = 100000
NS = N_FULL // NCORES  # 12500 rows per core
NT = 98  # n-tiles of 128 rows (12544 padded)
NTH = NT // 2  # 49 tiles per partition-half
NSP = NT * 128  # 12544
HALF_COLS = NTH * 128  # 6272
BHALF = 1024  # b chunk per PSUM cross tile / ACT instruction
CHUNK_TILES = 7  # dst DMA chunk granularity (7 column-blocks = 896 cols)
N_CHUNKS = NTH // CHUNK_TILES  # 7


def _pack_core_inputs(z, dataset, alpha):
    """Host-side packing: returns (in_maps, w) where w[b] = exp(-0.5*||z_b||^2)."""
    z = np.ascontiguousarray(z, dtype=np.float32)
    dataset = np.ascontiguousarray(dataset, dtype=np.float32)
    alpha = np.ascontiguousarray(alpha, dtype=np.float32)

    import ml_dtypes

    zT = z.T  # [64, B]
    zt_packed = np.concatenate([zT, zT], axis=0).astype(np.float16)  # [128, B]
    z_sq = np.sum(z.astype(np.float64) ** 2, axis=1)
    w = np.exp(-0.5 * z_sq)  # [B], applied on host at the end

    in_maps = []
    for c in range(NCORES):
        ds_c = dataset[c * NS : (c + 1) * NS]
        al_c = alpha[c * NS : (c + 1) * NS]
        dsp = np.zeros((NSP, D), np.float32)
        dsp[:NS] = ds_c
        alp = np.zeros((NSP, F), np.float32)
        alp[:NS] = al_c

        dsT = dsp.T  # [64, NSP]
        dst_packed = np.concatenate(
            [dsT[:, :HALF_COLS], dsT[:, HALF_COLS:]], axis=0
        ).astype(np.float16)  # [128, 6272]
        xsq_packed = np.ascontiguousarray(
            (-0.5 * np.sum(dsp * dsp, axis=1)).reshape(NT, 128).T
        )  # [128, NT]; column k = bias for tile k
        alp_packed = np.ascontiguousarray(
            alp.reshape(NT, 128, F).transpose(1, 0, 2).reshape(128, NT * F)
        ).astype(ml_dtypes.bfloat16)  # [128, NT*F]

        in_maps.append(
            {
                "zt": np.ascontiguousarray(zt_packed),
                "dst": np.ascontiguousarray(dst_packed),
                "alp": alp_packed,
                "xsq": xsq_packed,
            }
        )
    return in_maps, w


def build_nc(nt=NT):
    """Build the Bass module. nt can be reduced for simulator smoke tests."""
    import concourse.bass as bass
    import concourse.tile as tile
    from concourse import bacc, mybir

    assert nt % 2 == 0
    nth = nt // 2
    half_cols = nth * 128

    f32 = mybir.dt.float32
    f16 = mybir.dt.float16
    bf16 = mybir.dt.bfloat16

    nc = bacc.Bacc("TRN2", target_bir_lowering=False, debug=False)
    zt_d = nc.dram_tensor("zt", [128, B], f16, kind="ExternalInput").ap()
    dst_d = nc.dram_tensor("dst", [128, half_cols], f16, kind="ExternalInput").ap()
    alp_d = nc.dram_tensor("alp", [128, nt * F], bf16, kind="ExternalInput").ap()
    xsq_d = nc.dram_tensor("xsq", [128, nt], f32, kind="ExternalInput").ap()
    out_d = nc.dram_tensor("out", [F, B], f32, kind="ExternalOutput").ap()

    # dst DMA chunking (overlap load with compute)
    chunk_tiles = CHUNK_TILES if nth % CHUNK_TILES == 0 else 1
    n_chunks = nth // chunk_tiles
    chunk_cols = chunk_tiles * 128

    with tile.TileContext(nc) as tc:
        with (
            tc.tile_pool(name="consts", bufs=1) as consts,
            tc.tile_pool(name="g", bufs=3) as gpool,
            tc.tile_pool(name="ps_cross", bufs=1, space="PSUM") as ps_cross,
            tc.tile_pool(name="ps_acc", bufs=1, space="PSUM") as ps_acc,
        ):
            zt_sb = consts.tile([128, B], f16, tag="zt")
            nc.sync.dma_start(out=zt_sb, in_=zt_d)
            alp_sb = consts.tile([128, nt * F], bf16, tag="alp")
            nc.sync.dma_start(out=alp_sb, in_=alp_d)
            xsq_sb = consts.tile([128, nt], f32, tag="xsq")
            nc.sync.dma_start(out=xsq_sb, in_=xsq_d)
            dst_sb = []
            for j in range(n_chunks):
                t = consts.tile([128, chunk_cols], f16, tag=f"dst{j}")
                nc.sync.dma_start(
                    out=t, in_=dst_d[:, j * chunk_cols : (j + 1) * chunk_cols]
                )
                dst_sb.append(t)

            # 4 persistent accumulators: (b-half, 512-sub) -> [128, 512] PSUM
            # bank; col-group j (= k % 4) writes partitions 32j..32j+16.
            acc_ps = [
                ps_acc.tile([128, 512], f32, tag=f"acc{i}", name=f"acc{i}")
                for i in range(4)
            ]

            # Emission order of n-tiles: row-tiled pairs (p, nth+p); col-group
            # start/stop flags follow first/last occurrence in this order.
            order = []
            for p in range(nth):
                order += [p, nth + p]
            first_k, last_k = {}, {}
            for k in order:
                g_id = k % 4
                first_k.setdefault(g_id, k)
                last_k[g_id] = k

            for p in range(nth):
                chunk = dst_sb[p // chunk_tiles]
                coff = (p % chunk_tiles) * 128
                ks = (p, nth + p)
                lhss = (
                    chunk[0:64, coff : coff + 128],
                    chunk[64:128, coff : coff + 128],
                )
                for bh in (0, 1):
                    # row-tiled cross: both halves' MMs interleaved so they
                    # overlap in disjoint row groups of the PE array.
                    psA = ps_cross.tile([128, BHALF], f32, tag="crossA", name="psA")
                    psB = ps_cross.tile([128, BHALF], f32, tag="crossB", name="psB")
                    for s in (0, 1):
                        for h, pst in ((0, psA), (1, psB)):
                            nc.tensor.matmul(
                                pst[:, s * 512 : (s + 1) * 512],
                                lhsT=lhss[h],
                                rhs=zt_sb[
                                    h * 64 : (h + 1) * 64,
                                    bh * BHALF + s * 512 : bh * BHALF + (s + 1) * 512,
                                ],
                                start=True,
                                stop=True,
                            )
                    gs = []
                    for h, pst in ((0, psA), (1, psB)):
                        g = gpool.tile([128, BHALF], bf16, tag=f"g{h}", name=f"g{h}")
                        nc.scalar.activation(
                            out=g,
                            in_=pst,
                            func=mybir.ActivationFunctionType.Exp,
                            bias=xsq_sb[:, ks[h] : ks[h] + 1],
                            scale=1.0,
                        )
                        gs.append(g)
                    # col-tiled acc: adjacent MMs hit different col groups.
                    for s in (0, 1):
                        for h in (0, 1):
                            k = ks[h]
                            j = k % 4
                            nc.tensor.matmul(
                                acc_ps[bh * 2 + s][32 * j : 32 * j + F, :],
                                lhsT=alp_sb[:, k * F : (k + 1) * F],
                                rhs=gs[h][:, s * 512 : (s + 1) * 512],
                                start=(k == first_k[j]),
                                stop=(k == last_k[j]),
                                tile_position=(0, 32 * j),
                            )

            out_sb = consts.tile([F, B], f32, tag="out")
            for i in range(4):
                seg = out_sb[:, i * 512 : (i + 1) * 512]
                nc.vector.tensor_copy(out=seg, in_=acc_ps[i][0:F, :])
                for j in range(1, 4):
                    nc.vector.tensor_add(
                        out=seg, in0=seg, in1=acc_ps[i][32 * j : 32 * j + F, :]
                    )
            nc.sync.dma_start(out=out_d, in_=out_sb)

    nc.compile()
    return nc


def run_on_cores(in_maps, trace=False, **kwargs):
    from concourse.bass_utils import run_bass_kernel_spmd

    nc = build_nc()
    return run_bass_kernel_spmd(
        nc, in_maps, core_ids=list(range(NCORES)), trace=trace, **kwargs
    )


def kernel(z, dataset, alpha):
    in_maps, w = _pack_core_inputs(z, dataset, alpha)
    res = run_on_cores(in_maps, trace=False)
    total = np.zeros((F, B), np.float64)
    for r in res.results:
        total += r["out"].astype(np.float64)
    total *= w[None, :]
    return np.ascontiguousarray(total.T.astype(np.float32))


# revision 15
# speedup vs baseline: 1.8476x; 1.8476x over previous
"""RBF kernel regression (Gauss transform) on 8 Trainium2 NeuronCores.

Computes out = K @ alpha where K[b, n] = exp(-||z_b - x_n||^2 / 2),
z: [2048, 64], dataset: [100000, 64], alpha: [100000, 16].

Strategy (sharding_hint): shard dataset/alpha row-wise (N) across 8 cores.
Factorize K = exp(z.x) * exp(-x^2/2) * exp(-z^2/2): fold exp(-x^2/2) into
alpha on the host, apply exp(-z^2/2) on the host at the end. Each core then
computes partial[f, b] = sum_n alpha'[n, f] * exp(z.x_n) over its shard.

Per-core device pipeline (operands pre-packed/transposed on host):
  per (tile-pair, b-half) unit:
    cross kt/kb = dsT^T @ zT   (TensorE fp16, row-tiled pair: h0 + h64)
    G = exp(cross)             (ScalarE, no bias -> pure 1024-wide exps)
    AC = alpha'^T @ G          (TensorE bf16, col-tiled pair q0 + q32,
                                single-shot into a transient PSUM slot)
    acc_sb += AC               (VectorE band adds; PSUM slots rotate x4)
"""

import sys

if "/opt/trn_rl_repo" not in sys.path:
    sys.path.insert(0, "/opt/trn_rl_repo")

import numpy as np

B = 2048  # batch (queries)
D = 64  # feature dim
F = 16  # output dim
NCORES = 8
N_FULL = 100000
NS = N_FULL // NCORES  # 12500 rows per core
NT = 98  # n-tiles of 128 rows (12544 padded)
NTH = NT // 2  # 49 tiles per partition-half
NSP = NT * 128  # 12544
HALF_COLS = NTH * 128  # 6272
BHALF = 1024  # b chunk per PSUM slot / ACT instruction
CHUNK_TILES = 7  # dst DMA chunk granularity (7 column-blocks = 896 cols)


def _pack_core_inputs(z, dataset, alpha):
    """Host-side packing: returns (in_maps, w) where w[b] = exp(-0.5*||z_b||^2)."""
    import ml_dtypes

    z = np.ascontiguousarray(z, dtype=np.float32)
    dataset = np.ascontiguousarray(dataset, dtype=np.float32)
    alpha = np.ascontiguousarray(alpha, dtype=np.float32)

    zT = z.T  # [64, B]
    zt_packed = np.concatenate([zT, zT], axis=0).astype(np.float16)  # [128, B]
    z_sq = np.sum(z.astype(np.float64) ** 2, axis=1)
    w = np.exp(-0.5 * z_sq)  # [B], applied on host at the end

    in_maps = []
    for c in range(NCORES):
        ds_c = dataset[c * NS : (c + 1) * NS]
        al_c = alpha[c * NS : (c + 1) * NS]
        dsp = np.zeros((NSP, D), np.float32)
        dsp[:NS] = ds_c
        alp = np.zeros((NSP, F), np.float32)
        alp[:NS] = al_c
        # fold exp(-x^2/2) into alpha (float64 to keep tiny magnitudes exact)
        xsq = np.sum(dsp.astype(np.float64) ** 2, axis=1)
        alp = (alp.astype(np.float64) * np.exp(-0.5 * xsq)[:, None]).astype(
            np.float32
        )

        dsT = dsp.T  # [64, NSP]
        dst_packed = np.concatenate(
            [dsT[:, :HALF_COLS], dsT[:, HALF_COLS:]], axis=0
        ).astype(np.float16)  # [128, 6272]
        alp_packed = np.ascontiguousarray(
            alp.reshape(NT, 128, F).transpose(1, 0, 2).reshape(128, NT * F)
        ).astype(ml_dtypes.bfloat16)  # [128, NT*F]

        in_maps.append(
            {
                "zt": np.ascontiguousarray(zt_packed),
                "dst": np.ascontiguousarray(dst_packed),
                "alp": alp_packed,
            }
        )
    return in_maps, w


def build_nc(nt=NT):
    """Build the Bass module. nt can be reduced for simulator smoke tests."""
    import concourse.bass as bass
    import concourse.tile as tile
    from concourse import bacc, mybir

    assert nt % 2 == 0
    nth = nt // 2
    half_cols = nth * 128

    f32 = mybir.dt.float32
    f16 = mybir.dt.float16
    bf16 = mybir.dt.bfloat16

    nc = bacc.Bacc("TRN2", target_bir_lowering=False, debug=False)
    zt_d = nc.dram_tensor("zt", [128, B], f16, kind="ExternalInput").ap()
    dst_d = nc.dram_tensor("dst", [128, half_cols], f16, kind="ExternalInput").ap()
    alp_d = nc.dram_tensor("alp", [128, nt * F], bf16, kind="ExternalInput").ap()
    out_d = nc.dram_tensor("out", [64, B], f32, kind="ExternalOutput").ap()

    chunk_tiles = CHUNK_TILES if nth % CHUNK_TILES == 0 else 1
    n_chunks = nth // chunk_tiles
    chunk_cols = chunk_tiles * 128

    with tile.TileContext(nc) as tc:
        with (
            tc.tile_pool(name="consts", bufs=1) as consts,
            tc.tile_pool(name="g", bufs=3) as gpool,
            tc.tile_pool(name="ps_x", bufs=3, space="PSUM") as ps_x,
            tc.tile_pool(name="ps_acc", bufs=1, space="PSUM") as ps_acc,
        ):
            zt_sb = consts.tile([128, B], f16, tag="zt")
            nc.sync.dma_start(out=zt_sb, in_=zt_d)
            alp_sb = consts.tile([128, nt * F], bf16, tag="alp")
            nc.sync.dma_start(out=alp_sb, in_=alp_d)
            dst_sb = []
            for j in range(n_chunks):
                t = consts.tile([128, chunk_cols], f16, tag=f"dst{j}")
                nc.sync.dma_start(
                    out=t, in_=dst_d[:, j * chunk_cols : (j + 1) * chunk_cols]
                )
                dst_sb.append(t)

            out_sb = consts.tile([64, B], f32, tag="out")

            for bq in range(4):
                bs = bq * 512
                acc_t = ps_acc.tile([128, 512], f32, tag="acct", name="acct")
                acc_b = ps_acc.tile([128, 512], f32, tag="accb", name="accb")
                for p in range(nth):
                    chunk = dst_sb[p // chunk_tiles]
                    coff = (p % chunk_tiles) * 128
                    kt, kb = p, nth + p
                    # kt|kb interleaved in one PSUM tile: paired row-tiled MMs
                    x = ps_x.tile([128, BHALF], f32, tag="x", name="x")
                    nc.tensor.matmul(
                        x[:, 0:512],
                        lhsT=chunk[0:64, coff : coff + 128],
                        rhs=zt_sb[0:64, bs : bs + 512],
                        start=True,
                        stop=True,
                    )
                    nc.tensor.matmul(
                        x[:, 512:1024],
                        lhsT=chunk[64:128, coff : coff + 128],
                        rhs=zt_sb[64:128, bs : bs + 512],
                        start=True,
                        stop=True,
                    )
                    g = gpool.tile([128, BHALF], bf16, tag="g", name="g")
                    nc.scalar.activation(
                        out=g, in_=x, func=mybir.ActivationFunctionType.Exp
                    )
                    # paired col-tiled acc MMs into persistent accumulator
                    nc.tensor.matmul(
                        acc_t[0:F, :],
                        lhsT=alp_sb[:, kt * F : (kt + 1) * F],
                        rhs=g[:, 0:512],
                        start=(p == 0),
                        stop=(p == nth - 1),
                        tile_position=(0, 0),
                    )
                    nc.tensor.matmul(
                        acc_b[32 : 32 + F, :],
                        lhsT=alp_sb[:, kb * F : (kb + 1) * F],
                        rhs=g[:, 512:1024],
                        start=(p == 0),
                        stop=(p == nth - 1),
                        tile_position=(0, 32),
                    )
                nc.vector.tensor_copy(
                    out=out_sb[0:F, bs : bs + 512], in_=acc_t[0:F, :]
                )
                nc.vector.tensor_copy(
                    out=out_sb[32 : 32 + F, bs : bs + 512], in_=acc_b[32 : 32 + F, :]
                )

            nc.sync.dma_start(out=out_d[0:F, :], in_=out_sb[0:F, :])
            nc.sync.dma_start(
                out=out_d[32 : 32 + F, :], in_=out_sb[32 : 32 + F, :]
            )

    nc.compile()
    return nc


def run_on_cores(in_maps, trace=False, **kwargs):
    from concourse.bass_utils import run_bass_kernel_spmd

    nc = build_nc()
    return run_bass_kernel_spmd(
        nc, in_maps, core_ids=list(range(NCORES)), trace=trace, **kwargs
    )


def kernel(z, dataset, alpha):
    in_maps, w = _pack_core_inputs(z, dataset, alpha)
    res = run_on_cores(in_maps, trace=False)
    total = np.zeros((F, B), np.float64)
    for r in res.results:
        o = r["out"].astype(np.float64)  # [64, B]
        total += o[0:F] + o[32 : 32 + F]
    total *= w[None, :]
    return np.ascontiguousarray(total.T.astype(np.float32))


# revision 16
# speedup vs baseline: 1.8492x; 1.0009x over previous
"""RBF kernel regression (Gauss transform) on 8 Trainium2 NeuronCores.

Computes out = K @ alpha where K[b, n] = exp(-||z_b - x_n||^2 / 2),
z: [2048, 64], dataset: [100000, 64], alpha: [100000, 16].

Strategy (sharding_hint): shard dataset/alpha row-wise (N) across 8 cores.
Factorize K = exp(z.x) * exp(-x^2/2) * exp(-z^2/2): fold exp(-x^2/2) into
alpha on the host, apply exp(-z^2/2) on the host at the end. Each core then
computes partial[f, b] = sum_n alpha'[n, f] * exp(z.x_n) over its shard.

Per-core device pipeline (operands pre-packed/transposed on host):
  per (tile-pair, b-half) unit:
    cross kt/kb = dsT^T @ zT   (TensorE fp16, row-tiled pair: h0 + h64)
    G = exp(cross)             (ScalarE, no bias -> pure 1024-wide exps)
    AC = alpha'^T @ G          (TensorE bf16, col-tiled pair q0 + q32,
                                single-shot into a transient PSUM slot)
    acc_sb += AC               (VectorE band adds; PSUM slots rotate x4)
"""

import sys

if "/opt/trn_rl_repo" not in sys.path:
    sys.path.insert(0, "/opt/trn_rl_repo")

import numpy as np

B = 2048  # batch (queries)
D = 64  # feature dim
F = 16  # output dim
NCORES = 8
N_FULL = 100000
NS = N_FULL // NCORES  # 12500 rows per core
NT = 98  # n-tiles of 128 rows (12544 padded)
NTH = NT // 2  # 49 tiles per partition-half
NSP = NT * 128  # 12544
HALF_COLS = NTH * 128  # 6272
BHALF = 1024  # b chunk per PSUM slot / ACT instruction
CHUNK_TILES = 7  # dst DMA chunk granularity (7 column-blocks = 896 cols)


def _pack_core_inputs(z, dataset, alpha):
    """Host-side packing: returns (in_maps, w) where w[b] = exp(-0.5*||z_b||^2)."""
    import ml_dtypes

    z = np.ascontiguousarray(z, dtype=np.float32)
    dataset = np.ascontiguousarray(dataset, dtype=np.float32)
    alpha = np.ascontiguousarray(alpha, dtype=np.float32)

    zT = z.T  # [64, B]
    zt_packed = np.concatenate([zT, zT], axis=0).astype(np.float16)  # [128, B]
    z_sq = np.sum(z.astype(np.float64) ** 2, axis=1)
    w = np.exp(-0.5 * z_sq)  # [B], applied on host at the end

    in_maps = []
    for c in range(NCORES):
        ds_c = dataset[c * NS : (c + 1) * NS]
        al_c = alpha[c * NS : (c + 1) * NS]
        dsp = np.zeros((NSP, D), np.float32)
        dsp[:NS] = ds_c
        alp = np.zeros((NSP, F), np.float32)
        alp[:NS] = al_c
        # fold exp(-x^2/2) into alpha (float64 to keep tiny magnitudes exact)
        xsq = np.sum(dsp.astype(np.float64) ** 2, axis=1)
        alp = (alp.astype(np.float64) * np.exp(-0.5 * xsq)[:, None]).astype(
            np.float32
        )

        dsT = dsp.T  # [64, NSP]
        dst_packed = np.concatenate(
            [dsT[:, :HALF_COLS], dsT[:, HALF_COLS:]], axis=0
        ).astype(np.float16)  # [128, 6272]
        # pair layout: cols [32p, 32p+16) = tile p (h0), [32p+16, 32p+32) = tile NTH+p
        a3 = alp.reshape(NT, 128, F).transpose(1, 0, 2)  # [128, NT, F]
        pairs = np.concatenate([a3[:, :NTH], a3[:, NTH:]], axis=2)  # [128, NTH, 2F]
        alp_packed = np.ascontiguousarray(pairs.reshape(128, NT * F)).astype(
            ml_dtypes.bfloat16
        )  # [128, NT*F]

        in_maps.append(
            {
                "zt": np.ascontiguousarray(zt_packed),
                "dst": np.ascontiguousarray(dst_packed),
                "alp": alp_packed,
            }
        )
    return in_maps, w


def build_nc(nt=NT):
    """Build the Bass module. nt can be reduced for simulator smoke tests."""
    import concourse.bass as bass
    import concourse.tile as tile
    from concourse import bacc, mybir

    assert nt % 2 == 0
    nth = nt // 2
    half_cols = nth * 128

    f32 = mybir.dt.float32
    f16 = mybir.dt.float16
    bf16 = mybir.dt.bfloat16

    nc = bacc.Bacc("TRN2", target_bir_lowering=False, debug=False)
    zt_d = nc.dram_tensor("zt", [128, B], f16, kind="ExternalInput").ap()
    dst_d = nc.dram_tensor("dst", [128, half_cols], f16, kind="ExternalInput").ap()
    alp_d = nc.dram_tensor("alp", [128, nt * F], bf16, kind="ExternalInput").ap()
    out_d = nc.dram_tensor("out", [64, B], f32, kind="ExternalOutput").ap()

    chunk_tiles = CHUNK_TILES if nth % CHUNK_TILES == 0 else 1
    n_chunks = nth // chunk_tiles
    chunk_cols = chunk_tiles * 128

    with tile.TileContext(nc) as tc:
        with (
            tc.tile_pool(name="consts", bufs=1) as consts,
            tc.tile_pool(name="g", bufs=3) as gpool,
            tc.tile_pool(name="ps_x", bufs=3, space="PSUM") as ps_x,
            tc.tile_pool(name="ps_acc", bufs=1, space="PSUM") as ps_acc,
        ):
            zt_sb = []
            for q in range(4):
                zq = consts.tile([128, 512], f16, tag=f"zt{q}", name=f"zt{q}")
                nc.sync.dma_start(out=zq, in_=zt_d[:, q * 512 : (q + 1) * 512])
                zt_sb.append(zq)
            dst_sb, alp_sb = [], []
            for j in range(n_chunks):
                t = consts.tile([128, chunk_cols], f16, tag=f"dst{j}")
                nc.sync.dma_start(
                    out=t, in_=dst_d[:, j * chunk_cols : (j + 1) * chunk_cols]
                )
                dst_sb.append(t)
                ac = chunk_tiles * 2 * F  # alpha cols per chunk (pair layout)
                ta = consts.tile([128, ac], bf16, tag=f"alp{j}", name=f"alp{j}")
                nc.sync.dma_start(out=ta, in_=alp_d[:, j * ac : (j + 1) * ac])
                alp_sb.append(ta)

            out_sb = consts.tile([64, B], f32, tag="out")

            for bq in range(4):
                bs = bq * 512
                acc_t = ps_acc.tile([128, 512], f32, tag="acct", name="acct")
                acc_b = ps_acc.tile([128, 512], f32, tag="accb", name="accb")
                for p in range(nth):
                    chunk = dst_sb[p // chunk_tiles]
                    coff = (p % chunk_tiles) * 128
                    kt, kb = p, nth + p
                    # kt|kb interleaved in one PSUM tile: paired row-tiled MMs
                    x = ps_x.tile([128, BHALF], f32, tag="x", name="x")
                    nc.tensor.matmul(
                        x[:, 0:512],
                        lhsT=chunk[0:64, coff : coff + 128],
                        rhs=zt_sb[bq][0:64, :],
                        start=True,
                        stop=True,
                    )
                    nc.tensor.matmul(
                        x[:, 512:1024],
                        lhsT=chunk[64:128, coff : coff + 128],
                        rhs=zt_sb[bq][64:128, :],
                        start=True,
                        stop=True,
                    )
                    g = gpool.tile([128, BHALF], bf16, tag="g", name="g")
                    nc.scalar.activation(
                        out=g, in_=x, func=mybir.ActivationFunctionType.Exp
                    )
                    # paired col-tiled acc MMs into persistent accumulator
                    nc.tensor.matmul(
                        acc_t[0:F, :],
                        lhsT=alp_sb[p // chunk_tiles][
                            :, (p % chunk_tiles) * 2 * F : (p % chunk_tiles) * 2 * F + F
                        ],
                        rhs=g[:, 0:512],
                        start=(p == 0),
                        stop=(p == nth - 1),
                        tile_position=(0, 0),
                    )
                    nc.tensor.matmul(
                        acc_b[32 : 32 + F, :],
                        lhsT=alp_sb[p // chunk_tiles][
                            :,
                            (p % chunk_tiles) * 2 * F
                            + F : (p % chunk_tiles) * 2 * F
                            + 2 * F,
                        ],
                        rhs=g[:, 512:1024],
                        start=(p == 0),
                        stop=(p == nth - 1),
                        tile_position=(0, 32),
                    )
                nc.vector.tensor_copy(
                    out=out_sb[0:F, bs : bs + 512], in_=acc_t[0:F, :]
                )
                nc.vector.tensor_copy(
                    out=out_sb[32 : 32 + F, bs : bs + 512], in_=acc_b[32 : 32 + F, :]
                )

            nc.sync.dma_start(out=out_d[0:F, :], in_=out_sb[0:F, :])
            nc.sync.dma_start(
                out=out_d[32 : 32 + F, :], in_=out_sb[32 : 32 + F, :]
            )

    nc.compile()
    return nc


def run_on_cores(in_maps, trace=False, **kwargs):
    from concourse.bass_utils import run_bass_kernel_spmd

    nc = build_nc()
    return run_bass_kernel_spmd(
        nc, in_maps, core_ids=list(range(NCORES)), trace=trace, **kwargs
    )


def kernel(z, dataset, alpha):
    in_maps, w = _pack_core_inputs(z, dataset, alpha)
    res = run_on_cores(in_maps, trace=False)
    total = np.zeros((F, B), np.float64)
    for r in res.results:
        o = r["out"].astype(np.float64)  # [64, B]
        total += o[0:F] + o[32 : 32 + F]
    total *= w[None, :]
    return np.ascontiguousarray(total.T.astype(np.float32))


# revision 17
# speedup vs baseline: 1.8579x; 1.0047x over previous
"""RBF kernel regression (Gauss transform) on 8 Trainium2 NeuronCores.

Computes out = K @ alpha where K[b, n] = exp(-||z_b - x_n||^2 / 2),
z: [2048, 64], dataset: [100000, 64], alpha: [100000, 16].

Strategy (sharding_hint): shard dataset/alpha row-wise (N) across 8 cores.
Factorize K = exp(z.x) * exp(-x^2/2) * exp(-z^2/2): fold exp(-x^2/2) into
alpha on the host, apply exp(-z^2/2) on the host at the end. Each core then
computes partial[f, b] = sum_n alpha'[n, f] * exp(z.x_n) over its shard.

Per-core device pipeline (operands pre-packed/transposed on host):
  per (tile-pair, b-half) unit:
    cross kt/kb = dsT^T @ zT   (TensorE fp16, row-tiled pair: h0 + h64)
    G = exp(cross)             (ScalarE, no bias -> pure 1024-wide exps)
    AC = alpha'^T @ G          (TensorE bf16, col-tiled pair q0 + q32,
                                single-shot into a transient PSUM slot)
    acc_sb += AC               (VectorE band adds; PSUM slots rotate x4)
"""

import sys

if "/opt/trn_rl_repo" not in sys.path:
    sys.path.insert(0, "/opt/trn_rl_repo")

import numpy as np

B = 2048  # batch (queries)
D = 64  # feature dim
F = 16  # output dim
NCORES = 8
N_FULL = 100000
NS = N_FULL // NCORES  # 12500 rows per core
NT = 98  # n-tiles of 128 rows (12544 padded)
NTH = NT // 2  # 49 tiles per partition-half
NSP = NT * 128  # 12544
HALF_COLS = NTH * 128  # 6272
BHALF = 1024  # b chunk per PSUM slot / ACT instruction
CHUNK_TILES = 7  # dst DMA chunk granularity (7 column-blocks = 896 cols)


def _pack_core_inputs(z, dataset, alpha):
    """Host-side packing: returns (in_maps, w) where w[b] = exp(-0.5*||z_b||^2)."""
    import ml_dtypes

    z = np.ascontiguousarray(z, dtype=np.float32)
    dataset = np.ascontiguousarray(dataset, dtype=np.float32)
    alpha = np.ascontiguousarray(alpha, dtype=np.float32)

    zT = z.T  # [64, B]
    zt_packed = np.concatenate([zT, zT], axis=0).astype(np.float16)  # [128, B]
    z_sq = np.sum(z.astype(np.float64) ** 2, axis=1)
    w = np.exp(-0.5 * z_sq)  # [B], applied on host at the end

    in_maps = []
    for c in range(NCORES):
        ds_c = dataset[c * NS : (c + 1) * NS]
        al_c = alpha[c * NS : (c + 1) * NS]
        dsp = np.zeros((NSP, D), np.float32)
        dsp[:NS] = ds_c
        alp = np.zeros((NSP, F), np.float32)
        alp[:NS] = al_c
        # fold exp(-x^2/2) into alpha (float64 to keep tiny magnitudes exact)
        xsq = np.sum(dsp.astype(np.float64) ** 2, axis=1)
        alp = (alp.astype(np.float64) * np.exp(-0.5 * xsq)[:, None]).astype(
            np.float32
        )

        dsT = dsp.T  # [64, NSP]
        dst_packed = np.concatenate(
            [dsT[:, :HALF_COLS], dsT[:, HALF_COLS:]], axis=0
        ).astype(np.float16)  # [128, 6272]
        # pair layout: cols [32p, 32p+16) = tile p (h0), [32p+16, 32p+32) = tile NTH+p
        a3 = alp.reshape(NT, 128, F).transpose(1, 0, 2)  # [128, NT, F]
        pairs = np.concatenate([a3[:, :NTH], a3[:, NTH:]], axis=2)  # [128, NTH, 2F]
        alp_packed = np.ascontiguousarray(pairs.reshape(128, NT * F)).astype(
            ml_dtypes.bfloat16
        )  # [128, NT*F]

        in_maps.append(
            {
                "zt": np.ascontiguousarray(zt_packed),
                "dst": np.ascontiguousarray(dst_packed),
                "alp": alp_packed,
            }
        )
    return in_maps, w


def build_nc(nt=NT):
    """Build the Bass module. nt can be reduced for simulator smoke tests."""
    import concourse.bass as bass
    import concourse.tile as tile
    from concourse import bacc, mybir

    assert nt % 2 == 0
    nth = nt // 2
    half_cols = nth * 128

    f32 = mybir.dt.float32
    f16 = mybir.dt.float16
    bf16 = mybir.dt.bfloat16

    nc = bacc.Bacc("TRN2", target_bir_lowering=False, debug=False)
    zt_d = nc.dram_tensor("zt", [128, B], f16, kind="ExternalInput").ap()
    dst_d = nc.dram_tensor("dst", [128, half_cols], f16, kind="ExternalInput").ap()
    alp_d = nc.dram_tensor("alp", [128, nt * F], bf16, kind="ExternalInput").ap()
    out_d = nc.dram_tensor("out", [64, B], f32, kind="ExternalOutput").ap()

    chunk_tiles = CHUNK_TILES if nth % CHUNK_TILES == 0 else 1
    n_chunks = nth // chunk_tiles
    chunk_cols = chunk_tiles * 128

    with tile.TileContext(nc) as tc:
        with (
            tc.tile_pool(name="consts", bufs=1) as consts,
            tc.tile_pool(name="g", bufs=3) as gpool,
            tc.tile_pool(name="ps_x", bufs=3, space="PSUM") as ps_x,
            tc.tile_pool(name="ps_acc", bufs=1, space="PSUM") as ps_acc,
        ):
            # First-needed DMAs first: zt0 + dst0 on sync, alp0 on scalar
            # (the second HWDGE ring) so unit 0 unblocks ASAP.
            ac = chunk_tiles * 2 * F  # alpha cols per chunk (pair layout)
            zt_sb = [
                consts.tile([128, 512], f16, tag=f"zt{q}", name=f"ztq{q}")
                for q in range(4)
            ]
            dst_sb = [
                consts.tile([128, chunk_cols], f16, tag=f"dst{j}", name=f"dstc{j}")
                for j in range(n_chunks)
            ]
            alp_sb = [
                consts.tile([128, ac], bf16, tag=f"alp{j}", name=f"alpc{j}")
                for j in range(n_chunks)
            ]
            nc.sync.dma_start(out=zt_sb[0], in_=zt_d[:, 0:512])
            nc.sync.dma_start(out=dst_sb[0], in_=dst_d[:, 0:chunk_cols])
            nc.scalar.dma_start(out=alp_sb[0], in_=alp_d[:, 0:ac])
            for j in range(1, n_chunks):
                nc.sync.dma_start(
                    out=dst_sb[j], in_=dst_d[:, j * chunk_cols : (j + 1) * chunk_cols]
                )
                nc.scalar.dma_start(
                    out=alp_sb[j], in_=alp_d[:, j * ac : (j + 1) * ac]
                )
            for q in range(1, 4):
                nc.sync.dma_start(
                    out=zt_sb[q], in_=zt_d[:, q * 512 : (q + 1) * 512]
                )

            out_sb = consts.tile([64, B], f32, tag="out")

            for bq in range(4):
                bs = bq * 512
                acc_t = ps_acc.tile([128, 512], f32, tag="acct", name="acct")
                acc_b = ps_acc.tile([128, 512], f32, tag="accb", name="accb")
                for p in range(nth):
                    chunk = dst_sb[p // chunk_tiles]
                    coff = (p % chunk_tiles) * 128
                    kt, kb = p, nth + p
                    # kt|kb interleaved in one PSUM tile: paired row-tiled MMs
                    x = ps_x.tile([128, BHALF], f32, tag="x", name="x")
                    nc.tensor.matmul(
                        x[:, 0:512],
                        lhsT=chunk[0:64, coff : coff + 128],
                        rhs=zt_sb[bq][0:64, :],
                        start=True,
                        stop=True,
                    )
                    nc.tensor.matmul(
                        x[:, 512:1024],
                        lhsT=chunk[64:128, coff : coff + 128],
                        rhs=zt_sb[bq][64:128, :],
                        start=True,
                        stop=True,
                    )
                    g = gpool.tile([128, BHALF], bf16, tag="g", name="g")
                    nc.scalar.activation(
                        out=g, in_=x, func=mybir.ActivationFunctionType.Exp
                    )
                    # paired col-tiled acc MMs into persistent accumulator
                    nc.tensor.matmul(
                        acc_t[0:F, :],
                        lhsT=alp_sb[p // chunk_tiles][
                            :, (p % chunk_tiles) * 2 * F : (p % chunk_tiles) * 2 * F + F
                        ],
                        rhs=g[:, 0:512],
                        start=(p == 0),
                        stop=(p == nth - 1),
                        tile_position=(0, 0),
                    )
                    nc.tensor.matmul(
                        acc_b[32 : 32 + F, :],
                        lhsT=alp_sb[p // chunk_tiles][
                            :,
                            (p % chunk_tiles) * 2 * F
                            + F : (p % chunk_tiles) * 2 * F
                            + 2 * F,
                        ],
                        rhs=g[:, 512:1024],
                        start=(p == 0),
                        stop=(p == nth - 1),
                        tile_position=(0, 32),
                    )
                nc.vector.tensor_copy(
                    out=out_sb[0:F, bs : bs + 512], in_=acc_t[0:F, :]
                )
                nc.vector.tensor_copy(
                    out=out_sb[32 : 32 + F, bs : bs + 512], in_=acc_b[32 : 32 + F, :]
                )

            nc.sync.dma_start(out=out_d[0:F, :], in_=out_sb[0:F, :])
            nc.sync.dma_start(
                out=out_d[32 : 32 + F, :], in_=out_sb[32 : 32 + F, :]
            )

    nc.compile()
    return nc


def run_on_cores(in_maps, trace=False, **kwargs):
    from concourse.bass_utils import run_bass_kernel_spmd

    nc = build_nc()
    return run_bass_kernel_spmd(
        nc, in_maps, core_ids=list(range(NCORES)), trace=trace, **kwargs
    )


def kernel(z, dataset, alpha):
    in_maps, w = _pack_core_inputs(z, dataset, alpha)
    res = run_on_cores(in_maps, trace=False)
    total = np.zeros((F, B), np.float64)
    for r in res.results:
        o = r["out"].astype(np.float64)  # [64, B]
        total += o[0:F] + o[32 : 32 + F]
    total *= w[None, :]
    return np.ascontiguousarray(total.T.astype(np.float32))


# revision 18
# speedup vs baseline: 1.8669x; 1.0048x over previous
"""RBF kernel regression (Gauss transform) on 8 Trainium2 NeuronCores.

Computes out = K @ alpha where K[b, n] = exp(-||z_b - x_n||^2 / 2),
z: [2048, 64], dataset: [100000, 64], alpha: [100000, 16].

Strategy (sharding_hint): shard dataset/alpha row-wise (N) across 8 cores.
Factorize K = exp(z.x) * exp(-x^2/2) * exp(-z^2/2): fold exp(-x^2/2) into
alpha on the host, apply exp(-z^2/2) on the host at the end. Each core then
computes partial[f, b] = sum_n alpha'[n, f] * exp(z.x_n) over its shard.

Per-core device pipeline (operands pre-packed/transposed on host):
  per (tile-pair, b-half) unit:
    cross kt/kb = dsT^T @ zT   (TensorE fp16, row-tiled pair: h0 + h64)
    G = exp(cross)             (ScalarE, no bias -> pure 1024-wide exps)
    AC = alpha'^T @ G          (TensorE bf16, col-tiled pair q0 + q32,
                                single-shot into a transient PSUM slot)
    acc_sb += AC               (VectorE band adds; PSUM slots rotate x4)
"""

import sys

if "/opt/trn_rl_repo" not in sys.path:
    sys.path.insert(0, "/opt/trn_rl_repo")

import numpy as np

B = 2048  # batch (queries)
D = 64  # feature dim
F = 16  # output dim
NCORES = 8
N_FULL = 100000
NS = N_FULL // NCORES  # 12500 rows per core
NT = 98  # n-tiles of 128 rows (12544 padded)
NTH = NT // 2  # 49 tiles per partition-half
NSP = NT * 128  # 12544
HALF_COLS = NTH * 128  # 6272
BHALF = 1024  # b chunk per PSUM slot / ACT instruction
CHUNK_TILES = 7  # dst DMA chunk granularity (7 column-blocks = 896 cols)


def _pack_core_inputs(z, dataset, alpha):
    """Host-side packing: returns (in_maps, w) where w[b] = exp(-0.5*||z_b||^2)."""
    import ml_dtypes

    z = np.ascontiguousarray(z, dtype=np.float32)
    dataset = np.ascontiguousarray(dataset, dtype=np.float32)
    alpha = np.ascontiguousarray(alpha, dtype=np.float32)

    zT = z.T  # [64, B]
    zt_packed = np.concatenate([zT, zT], axis=0).astype(np.float16)  # [128, B]
    z_sq = np.sum(z.astype(np.float64) ** 2, axis=1)
    w = np.exp(-0.5 * z_sq)  # [B], applied on host at the end

    in_maps = []
    for c in range(NCORES):
        ds_c = dataset[c * NS : (c + 1) * NS]
        al_c = alpha[c * NS : (c + 1) * NS]
        dsp = np.zeros((NSP, D), np.float32)
        dsp[:NS] = ds_c
        alp = np.zeros((NSP, F), np.float32)
        alp[:NS] = al_c
        # fold exp(-x^2/2) into alpha (float64 to keep tiny magnitudes exact)
        xsq = np.sum(dsp.astype(np.float64) ** 2, axis=1)
        alp = (alp.astype(np.float64) * np.exp(-0.5 * xsq)[:, None]).astype(
            np.float32
        )

        dsT = dsp.T  # [64, NSP]
        dst_packed = np.concatenate(
            [dsT[:, :HALF_COLS], dsT[:, HALF_COLS:]], axis=0
        ).astype(np.float16)  # [128, 6272]
        # pair layout: cols [32p, 32p+16) = tile p (h0), [32p+16, 32p+32) = tile NTH+p
        a3 = alp.reshape(NT, 128, F).transpose(1, 0, 2)  # [128, NT, F]
        pairs = np.concatenate([a3[:, :NTH], a3[:, NTH:]], axis=2)  # [128, NTH, 2F]
        alp_packed = np.ascontiguousarray(pairs.reshape(128, NT * F)).astype(
            ml_dtypes.bfloat16
        )  # [128, NT*F]

        in_maps.append(
            {
                "zt": np.ascontiguousarray(zt_packed),
                "dst": np.ascontiguousarray(dst_packed),
                "alp": alp_packed,
            }
        )
    return in_maps, w


def build_nc(nt=NT):
    """Build the Bass module. nt can be reduced for simulator smoke tests."""
    import concourse.bass as bass
    import concourse.tile as tile
    from concourse import bacc, mybir

    assert nt % 2 == 0
    nth = nt // 2
    half_cols = nth * 128

    f32 = mybir.dt.float32
    f16 = mybir.dt.float16
    bf16 = mybir.dt.bfloat16

    nc = bacc.Bacc("TRN2", target_bir_lowering=False, debug=False)
    zt_d = nc.dram_tensor("zt", [128, B], f16, kind="ExternalInput").ap()
    dst_d = nc.dram_tensor("dst", [128, half_cols], f16, kind="ExternalInput").ap()
    alp_d = nc.dram_tensor("alp", [128, nt * F], bf16, kind="ExternalInput").ap()
    out_d = nc.dram_tensor("out", [64, B], f32, kind="ExternalOutput").ap()

    chunk_tiles = CHUNK_TILES if nth % CHUNK_TILES == 0 else 1
    n_chunks = nth // chunk_tiles
    chunk_cols = chunk_tiles * 128

    with tile.TileContext(nc) as tc:
        with (
            tc.tile_pool(name="consts", bufs=1) as consts,
            tc.tile_pool(name="g", bufs=3) as gpool,
            tc.tile_pool(name="ps_x", bufs=3, space="PSUM") as ps_x,
            tc.tile_pool(name="ps_acc", bufs=1, space="PSUM") as ps_acc,
        ):
            # First-needed DMAs first: zt0 + dst0 on sync, alp0 on scalar
            # (the second HWDGE ring) so unit 0 unblocks ASAP.
            ac = chunk_tiles * 2 * F  # alpha cols per chunk (pair layout)
            zt_sb = [
                consts.tile([128, 512], f16, tag=f"zt{q}", name=f"ztq{q}")
                for q in range(4)
            ]
            dst_sb = [
                consts.tile([128, chunk_cols], f16, tag=f"dst{j}", name=f"dstc{j}")
                for j in range(n_chunks)
            ]
            alp_sb = [
                consts.tile([128, ac], bf16, tag=f"alp{j}", name=f"alpc{j}")
                for j in range(n_chunks)
            ]
            nc.sync.dma_start(out=zt_sb[0], in_=zt_d[:, 0:512])
            nc.sync.dma_start(out=dst_sb[0], in_=dst_d[:, 0:chunk_cols])
            nc.scalar.dma_start(out=alp_sb[0], in_=alp_d[:, 0:ac])
            for j in range(1, n_chunks):
                nc.sync.dma_start(
                    out=dst_sb[j], in_=dst_d[:, j * chunk_cols : (j + 1) * chunk_cols]
                )
                nc.sync.dma_start(out=alp_sb[j], in_=alp_d[:, j * ac : (j + 1) * ac])
            for q in range(1, 4):
                nc.sync.dma_start(
                    out=zt_sb[q], in_=zt_d[:, q * 512 : (q + 1) * 512]
                )

            out_sb = consts.tile([64, B], f32, tag="out")

            for bq in range(4):
                bs = bq * 512
                acc_t = ps_acc.tile([128, 512], f32, tag="acct", name="acct")
                acc_b = ps_acc.tile([128, 512], f32, tag="accb", name="accb")
                for p in range(nth):
                    chunk = dst_sb[p // chunk_tiles]
                    coff = (p % chunk_tiles) * 128
                    kt, kb = p, nth + p
                    # kt|kb interleaved in one PSUM tile: paired row-tiled MMs
                    x = ps_x.tile([128, BHALF], f32, tag="x", name="x")
                    nc.tensor.matmul(
                        x[:, 0:512],
                        lhsT=chunk[0:64, coff : coff + 128],
                        rhs=zt_sb[bq][0:64, :],
                        start=True,
                        stop=True,
                    )
                    nc.tensor.matmul(
                        x[:, 512:1024],
                        lhsT=chunk[64:128, coff : coff + 128],
                        rhs=zt_sb[bq][64:128, :],
                        start=True,
                        stop=True,
                    )
                    g = gpool.tile([128, BHALF], bf16, tag="g", name="g")
                    nc.scalar.activation(
                        out=g, in_=x, func=mybir.ActivationFunctionType.Exp
                    )
                    # paired col-tiled acc MMs into persistent accumulator
                    nc.tensor.matmul(
                        acc_t[0:F, :],
                        lhsT=alp_sb[p // chunk_tiles][
                            :, (p % chunk_tiles) * 2 * F : (p % chunk_tiles) * 2 * F + F
                        ],
                        rhs=g[:, 0:512],
                        start=(p == 0),
                        stop=(p == nth - 1),
                        tile_position=(0, 0),
                    )
                    nc.tensor.matmul(
                        acc_b[32 : 32 + F, :],
                        lhsT=alp_sb[p // chunk_tiles][
                            :,
                            (p % chunk_tiles) * 2 * F
                            + F : (p % chunk_tiles) * 2 * F
                            + 2 * F,
                        ],
                        rhs=g[:, 512:1024],
                        start=(p == 0),
                        stop=(p == nth - 1),
                        tile_position=(0, 32),
                    )
                nc.vector.tensor_copy(
                    out=out_sb[0:F, bs : bs + 512], in_=acc_t[0:F, :]
                )
                nc.vector.tensor_copy(
                    out=out_sb[32 : 32 + F, bs : bs + 512], in_=acc_b[32 : 32 + F, :]
                )

            nc.sync.dma_start(out=out_d[0:F, :], in_=out_sb[0:F, :])
            nc.sync.dma_start(
                out=out_d[32 : 32 + F, :], in_=out_sb[32 : 32 + F, :]
            )

    nc.compile()
    return nc


def run_on_cores(in_maps, trace=False, **kwargs):
    from concourse.bass_utils import run_bass_kernel_spmd

    nc = build_nc()
    return run_bass_kernel_spmd(
        nc, in_maps, core_ids=list(range(NCORES)), trace=trace, **kwargs
    )


def kernel(z, dataset, alpha):
    in_maps, w = _pack_core_inputs(z, dataset, alpha)
    res = run_on_cores(in_maps, trace=False)
    total = np.zeros((F, B), np.float64)
    for r in res.results:
        o = r["out"].astype(np.float64)  # [64, B]
        total += o[0:F] + o[32 : 32 + F]
    total *= w[None, :]
    return np.ascontiguousarray(total.T.astype(np.float32))
